# revision 40
# baseline (speedup 1.0000x reference)
"""Trainium2 Bass kernel for nn_CrossAttention (B=4, C=256, H=W=64).

reference:
    a_flat [B,C,Na], b_flat [B,C,Nb], W [C,C];  Na = Nb = 4096
    S[b,n,m]  = sum_{c,d} a[b,c,n] W[d,c] b[b,d,m]      (= Wa^T @ b, Wa = W @ a_flat)
    a_new     = a_flat @ softmax(S, axis=n)             -> [B,C,Nb]
    b_new     = b_flat @ softmax(S, axis=m)^T           -> [B,C,Na]

Sharding: 8 cores = 4 "a-cores" (batch i computes a_new[i]) + 4 "b-cores"
(batch i computes b_new[i]).  Both run the SAME device kernel:

    T[l,r]   = sum_d P[d,l] Q[d,r]          (l,r = 4096, d = 256)
    E[l,r]   = exp(T[l,r] - K)              (K fixed shift, cancels in ratio)
    OUT[r,c] = sum_l E[l,r] Z[l,c] / sum_l E[l,r]

a-core: P=Wa_i, Q=b_i, Z=a_i^T  ->  OUT = a_new_i^T
b-core: P=b_i, Q=Wa_i, Z=b_i^T  ->  OUT = b_new_i^T

The fixed shift K replaces the softmax max-subtraction: softmax is invariant
to any shift, so a per-column max is unnecessary as long as exp stays inside
fp32 range.  Here S ~ N(0,16^2) with |S|max ~ 96 and min per-column max ~ 33,
so K=64 keeps exp(T-K) within [e^-160, e^32] (no inf) and every column's
denominator far above underflow.

The softmax denominator comes for free as a 257th ones-column appended to Z.
Matmuls run in float32r (full fp32 at ~1 cycle/row for free dims >= 256).
"""

import numpy as np

_STATE = {}

P = 128
C = 256          # channels (contraction dim for T, output dim for OUT)
N = 4096         # Na = Nb
MB = 512         # m-block (free dim of S tiles; one PSUM bank)
NT = N // P      # 32 l-tiles
MBS = N // MB    # 8 r-blocks
KSHIFT = 64.0
HW_SHAPE = (64, 64)

# Partition-major DRAM layouts for z/out (bigger DMA descriptors).  Measured
# SLOWER (309us vs 184us): the coarse z slices starve the U accumulation,
# which consumes every z tile within the first m-block.  Keep False.
BIGDESC = False


def _build(
    reps=1,
    loop_trip=None,
    u_j=4,
    preload=False,
    lag=1,
    exp_split=True,
    s_only=False,
    u_only=False,
    staged_exp=False,
    dma_split=False,
    imm_bias=False,
):
    if staged_exp:
        lag = max(lag, 2)
    import contextlib

    import concourse.mybir as mybir
    import concourse.tile as tile
    from concourse import bacc
    from concourse.bass import ds, ts

    f32 = mybir.dt.float32
    f32r = mybir.dt.float32r
    bf16 = mybir.dt.bfloat16

    nc = bacc.Bacc("TRN2", target_bir_lowering=False)
    p_in = nc.dram_tensor("p_in", [C, N], f32r, kind="ExternalInput")
    q_in = nc.dram_tensor("q_in", [C, N], f32r, kind="ExternalInput")
    if BIGDESC:
        z_in = nc.dram_tensor("z_in", [P, NT, C + 2], bf16, kind="ExternalInput")
        out_t = nc.dram_tensor("out_t", [P, N // P, C], f32, kind="ExternalOutput")
    else:
        z_in = nc.dram_tensor("z_in", [N, C + 2], bf16, kind="ExternalInput")
        out_t = nc.dram_tensor("out_t", [N, C], f32, kind="ExternalOutput")

    ZG = 4  # z-load granularity (nt tiles per DMA)

    with tile.TileContext(nc) as tc:
        with (
            tc.tile_pool(name="big", bufs=1) as big,
            tc.tile_pool(name="epool", bufs=4) as epool,
            tc.tile_pool(name="stpool", bufs=3) as stpool,
            tc.tile_pool(name="opool", bufs=3) as opool,
            tc.tile_pool(name="small", bufs=4) as small,
            tc.tile_pool(name="spsum", bufs=2, space="PSUM") as spsum,
            tc.tile_pool(name="upsum", bufs=4, space="PSUM") as upsum,
        ):
            # Resident inputs.  p/q: [d, l|r] as [128, 2, N]; z: [l, c+pad]
            # as [128, NT, C+2] with two ones-columns (denominator + fp32r
            # even-width padding).  q and z are loaded in slices so the
            # first matmuls don't wait for the full 12 MB of input.
            p_t = big.tile([P, 2, N], f32r, tag="p", name="p_t")
            q_t = big.tile([P, 2, N], f32r, tag="q", name="q_t")
            z_t = big.tile([P, NT, C + 2], bf16, tag="z", name="z_t")
            kbias = small.tile([P, 1], f32, tag="kbias", name="kbias")
            nc.vector.memset(kbias[:], -KSHIFT)
            if u_only:
                e2c = big.tile([P, 2, MB], bf16, tag="e2c", name="e2c")
                nc.vector.memset(e2c[:], 0.001)

            p_src = p_in.rearrange("(ko p) n -> p ko n", p=P)
            q_src = q_in.rearrange("(ko p) n -> p ko n", p=P)
            if BIGDESC:
                z_src = z_in
            else:
                z_src = z_in.rearrange("(nt p) c -> p nt c", p=P)

            if loop_trip is not None:
                rep_ctx = lambda: tc.For_i(  # noqa: E731
                    0,
                    loop_trip,
                    1,
                    hint_engines=(
                        mybir.EngineType.PE,
                        mybir.EngineType.Activation,
                        mybir.EngineType.DVE,
                        mybir.EngineType.SP,
                    ),
                )
            else:
                rep_ctx = contextlib.nullcontext

            # issue order: what the first matmuls need comes first.
            # dma_split: q+z ride the Act HWDGE queue (issued at the
            # iteration head where their waits are already satisfied, so
            # they never stall the ACT compute stream); p + out stores
            # stay on the SP queue.
            ldq_eng = nc.scalar if dma_split else nc.sync

            def load_p(pg):
                nc.sync.dma_start(
                    p_t[:, :, ts(pg, N // 4)], p_src[:, :, ts(pg, N // 4)]
                )

            def load_q(mbq):
                ldq_eng.dma_start(q_t[:, :, ts(mbq, MB)], q_src[:, :, ts(mbq, MB)])

            def load_z(zg):
                ldq_eng.dma_start(z_t[:, ts(zg, ZG), :], z_src[:, ts(zg, ZG), :])

            def load_all():
                if BIGDESC:
                    # few DMAs with multi-KB per-partition descriptors; the
                    # head slices are small so the first matmuls start early
                    nc.sync.dma_start(p_t[:, :, ts(0, 512)], p_src[:, :, ts(0, 512)])
                    nc.sync.dma_start(q_t[:, :, ts(0, 512)], q_src[:, :, ts(0, 512)])
                    nc.sync.dma_start(z_t[:, 0:2, :], z_src[:, 0:2, :])
                    nc.sync.dma_start(
                        p_t[:, :, ds(512, N - 512)], p_src[:, :, ds(512, N - 512)]
                    )
                    for lo, sz in ((512, 1024), (1536, 1024), (2560, 1536)):
                        nc.sync.dma_start(
                            q_t[:, :, ds(lo, sz)], q_src[:, :, ds(lo, sz)]
                        )
                    for lo, sz in ((2, 6), (8, 12), (20, 12)):
                        nc.sync.dma_start(
                            z_t[:, ds(lo, sz), :], z_src[:, ds(lo, sz), :]
                        )
                    return
                load_p(0)
                load_q(0)
                load_z(0)
                for pg in range(1, 4):
                    load_p(pg)
                for i in range(1, MBS):
                    load_q(i)
                    load_z(i)

            if preload:
                load_all()

            def emit_body():
                if not preload:
                    load_all()

                for mb in range(MBS):
                    u_ps = [
                        upsum.tile([P, C + 2], mybir.dt.float32, tag="u", name=f"u{j}")
                        for j in range(u_j)
                    ]

                    def u_matmuls(nt2, e2_t):
                        for h in range(2):
                            nt = 2 * nt2 + h
                            for j in range(u_j):
                                nc.tensor.matmul(
                                    u_ps[j][:],
                                    e2_t[:, h, ts(j, P)],
                                    z_t[:, nt, :],
                                    start=(nt == 0),
                                    stop=(nt == NT - 1),
                                )

                    if s_only:
                        # probe: pure S-MM stream; a tiny DVE copy + DMA
                        # keeps each s tile live without ACT/U work
                        for nt2 in range(NT // 2):
                            s_ps = spsum.tile(
                                [P, 2, MB], mybir.dt.float32, tag="s", name="s"
                            )
                            for h in range(2):
                                nt = 2 * nt2 + h
                                for ko in range(2):
                                    nc.tensor.matmul(
                                        s_ps[:, h, :],
                                        p_t[:, ko, ts(nt, P)],
                                        q_t[:, ko, ts(mb, MB)],
                                        start=(ko == 0),
                                        stop=(ko == 1),
                                    )
                            o_t = opool.tile([P, 32], f32, tag="o", name="o")
                            nc.vector.tensor_copy(o_t[:], s_ps[:, 0, 0:32])
                            blk = (mb * (NT // 2) + nt2) % 32
                            if BIGDESC:
                                nc.sync.dma_start(out_t[:, blk, 0:32], o_t[:])
                            else:
                                nc.sync.dma_start(
                                    out_t[ds(blk * P, P), 0:32], o_t[:]
                                )
                        continue

                    if u_only:
                        # probe: pure U-MM stream off a constant E tile
                        for nt2 in range(NT // 2):
                            u_matmuls(nt2, e2c)
                        o4_t = opool.tile([P, u_j, C], f32, tag="o", name="o")
                        for j in range(u_j):
                            recip = small.tile([P, 1], f32, tag="recip", name="recip")
                            nc.vector.reciprocal(recip[:], u_ps[j][:, C : C + 1])
                            nc.vector.tensor_scalar_mul(
                                o4_t[:, j, :], u_ps[j][:, 0:C], recip[:]
                            )
                        if BIGDESC:
                            nc.sync.dma_start(
                                out_t[:, ds(mb * u_j, u_j), :], o4_t[:]
                            )
                        else:
                            for j in range(u_j):
                                nc.sync.dma_start(
                                    out_t[ds(mb * MB + j * P, P), :], o4_t[:, j, :]
                                )
                        continue

                    # Two S tiles share one PSUM tile (2 banks) so a single
                    # ACT instruction computes exp over 1024 elems/partition,
                    # halving ACT fixed overhead.  Software-pipelined with a
                    # lag of 2: PE runs S(nt2) before U(nt2-2), giving each
                    # exp two full PE iterations of slack so the U matmuls
                    # never wait on ACT.
                    pending = []
                    for nt2 in range(NT // 2):
                        s_ps = spsum.tile(
                            [P, 2, MB], mybir.dt.float32, tag="s", name="s"
                        )
                        for h in range(2):
                            nt = 2 * nt2 + h
                            for ko in range(2):
                                nc.tensor.matmul(
                                    s_ps[:, h, :],
                                    p_t[:, ko, ts(nt, P)],
                                    q_t[:, ko, ts(mb, MB)],
                                    start=(ko == 0),
                                    stop=(ko == 1),
                                )
                        e2_t = epool.tile([P, 2, MB], bf16, tag="e", name="e")
                        if staged_exp:
                            # DVE stages S into SBUF so ACT can run one
                            # full-rate N=1024 exp (ACT PSUM reads are only
                            # full-rate within a single 2KB bank).
                            st_t = stpool.tile([P, 2, MB], f32, tag="st", name="st")
                            nc.vector.tensor_copy(st_t[:], s_ps[:])
                            nc.scalar.activation(
                                e2_t[:],
                                st_t[:],
                                mybir.ActivationFunctionType.Exp,
                                bias=kbias[:],
                            )
                        elif exp_split:
                            for h in range(2):
                                nc.scalar.activation(
                                    e2_t[:, h, :],
                                    s_ps[:, h, :],
                                    mybir.ActivationFunctionType.Exp,
                                    bias=(-KSHIFT) if imm_bias else kbias[:],
                                )
                        else:
                            nc.scalar.activation(
                                e2_t[:],
                                s_ps[:],
                                mybir.ActivationFunctionType.Exp,
                                bias=kbias[:],
                            )
                        pending.append((nt2, e2_t))
                        if len(pending) > lag:
                            u_matmuls(*pending.pop(0))
                    for args in pending:
                        u_matmuls(*args)

                    if BIGDESC:
                        o4_t = opool.tile([P, u_j, C], f32, tag="o", name="o")
                        for j in range(u_j):
                            recip = small.tile([P, 1], f32, tag="recip", name="recip")
                            nc.vector.reciprocal(recip[:], u_ps[j][:, C : C + 1])
                            nc.vector.tensor_scalar_mul(
                                o4_t[:, j, :], u_ps[j][:, 0:C], recip[:]
                            )
                        nc.sync.dma_start(
                            out_t[:, ds(mb * u_j, u_j), :], o4_t[:]
                        )
                    else:
                        for j in range(u_j):
                            recip = small.tile([P, 1], f32, tag="recip", name="recip")
                            nc.vector.reciprocal(recip[:], u_ps[j][:, C : C + 1])
                            o_t = opool.tile([P, C], f32, tag="o", name="o")
                            nc.vector.tensor_scalar_mul(
                                o_t[:], u_ps[j][:, 0:C], recip[:]
                            )
                            nc.sync.dma_start(
                                out_t[ds(mb * MB + j * P, P), :], o_t[:]
                            )

            if loop_trip is not None:
                with rep_ctx():
                    for _rep in range(reps):
                        emit_body()
            else:
                for _rep in range(reps):
                    emit_body()

    nc.compile()
    return nc


def _get_nc(reps=1, loop_trip=None, **bkw):
    key = f"nc{reps}_{loop_trip}_{sorted(bkw.items())}"
    if key not in _STATE:
        _STATE[key] = _build(reps, loop_trip, **bkw)
    return _STATE[key]


def _with_ones(x):
    import ml_dtypes

    z = np.ones((N, C + 2), dtype=ml_dtypes.bfloat16)
    z[:, 0:C] = x.T.astype(ml_dtypes.bfloat16)
    if BIGDESC:
        # partition-major: z[p, nt, c] = z_rowmajor[nt*128 + p, c]
        z = np.ascontiguousarray(z.reshape(NT, P, C + 2).transpose(1, 0, 2))
    return z


def _prep_inputs(a, b, W):
    a = np.asarray(a, dtype=np.float32)
    b = np.asarray(b, dtype=np.float32)
    W = np.asarray(W, dtype=np.float32)
    B = a.shape[0]
    af = a.reshape(B, C, N)
    bf = b.reshape(B, C, N)
    Wa = np.matmul(W[None], af)  # [B, C, N]
    in_maps = []
    for i in range(B):  # a-cores
        in_maps.append(
            {
                "p_in": np.ascontiguousarray(Wa[i]),
                "q_in": np.ascontiguousarray(bf[i]),
                "z_in": _with_ones(af[i]),
            }
        )
    for i in range(B):  # b-cores
        in_maps.append(
            {
                "p_in": np.ascontiguousarray(bf[i]),
                "q_in": np.ascontiguousarray(Wa[i]),
                "z_in": _with_ones(bf[i]),
            }
        )
    return in_maps


def _unout(r):
    # [P, N//P, C] partition-major -> [N, C] row-major -> [C, N]
    if BIGDESC:
        return r.swapaxes(0, 1).reshape(N, C).T
    return r.T


def _postprocess(results, B):
    a_new = np.stack([_unout(results[i]["out_t"]) for i in range(B)])
    b_new = np.stack([_unout(results[B + i]["out_t"]) for i in range(B)])
    a_new = np.ascontiguousarray(a_new).reshape(B, C, *HW_SHAPE)
    b_new = np.ascontiguousarray(b_new).reshape(B, C, *HW_SHAPE)
    return a_new, b_new


def _run(a, b, W, reps=1, loop_trip=None, **run_kwargs):
    from concourse.bass_utils import run_bass_kernel_spmd

    in_maps = _prep_inputs(a, b, W)
    nc = _get_nc(reps, loop_trip)
    res = run_bass_kernel_spmd(nc, in_maps, core_ids=list(range(len(in_maps))), **run_kwargs)
    return _postprocess(res.results, len(in_maps) // 2), res


def kernel(a, b, W):
    (a_new, b_new), _ = _run(a, b, W)
    return a_new, b_new



# revision 43
# speedup vs baseline: 1.0687x; 1.0687x over previous
"""Trainium2 Bass kernel for nn_CrossAttention (B=4, C=256, H=W=64).

reference:
    a_flat [B,C,Na], b_flat [B,C,Nb], W [C,C];  Na = Nb = 4096
    S[b,n,m]  = sum_{c,d} a[b,c,n] W[d,c] b[b,d,m]      (= Wa^T @ b, Wa = W @ a_flat)
    a_new     = a_flat @ softmax(S, axis=n)             -> [B,C,Nb]
    b_new     = b_flat @ softmax(S, axis=m)^T           -> [B,C,Na]

Sharding: 8 cores = 4 "a-cores" (batch i computes a_new[i]) + 4 "b-cores"
(batch i computes b_new[i]).  Both run the SAME device kernel:

    T[l,r]   = sum_d P[d,l] Q[d,r]          (l,r = 4096, d = 256)
    E[l,r]   = exp(T[l,r] - K)              (K fixed shift, cancels in ratio)
    OUT[r,c] = sum_l E[l,r] Z[l,c] / sum_l E[l,r]

a-core: P=Wa_i, Q=b_i, Z=a_i^T  ->  OUT = a_new_i^T
b-core: P=b_i, Q=Wa_i, Z=b_i^T  ->  OUT = b_new_i^T

The fixed shift K replaces the softmax max-subtraction: softmax is invariant
to any shift, so a per-column max is unnecessary as long as exp stays inside
fp32 range.  Here S ~ N(0,16^2) with |S|max ~ 96 and min per-column max ~ 33,
so K=64 keeps exp(T-K) within [e^-160, e^32] (no inf) and every column's
denominator far above underflow.

The softmax denominator comes for free as a 257th ones-column appended to Z.
S matmuls run in float32r (full fp32, needed because the softmax is extremely
peaked: S ~ N(0,16^2), so bf16-level S error flips near-tied argmax columns).
The U path (E and Z) runs in bf16: the post-softmax weights only enter
linearly, so bf16's ~0.4% error is harmless, and bf16 stationary tiles get
fast weight loads on the PE.

exp runs as two N=512 ACT instructions per S tile pair (exp_split): ACT reads
PSUM at full rate only within a single 2KB bank; one N=1024 instruction
spanning two banks measured ~2000ns vs 2x719ns split.

Measured body time (loop_trip harness, 8 cores): 302us baseline -> 184-267us
depending on device power state (a late-session ~20% slowdown, consistent
with P0 downclock, affected all configs uniformly).
"""

import numpy as np

_STATE = {}

P = 128
C = 256          # channels (contraction dim for T, output dim for OUT)
N = 4096         # Na = Nb
MB = 512         # m-block (free dim of S tiles; one PSUM bank)
NT = N // P      # 32 l-tiles
MBS = N // MB    # 8 r-blocks
KSHIFT = 64.0
HW_SHAPE = (64, 64)

# Partition-major DRAM layouts for z/out (bigger DMA descriptors).  Measured
# SLOWER (309us vs 184us): the coarse z slices starve the U accumulation,
# which consumes every z tile within the first m-block.  Keep False.
BIGDESC = False


def _build(
    reps=1,
    loop_trip=None,
    u_j=4,
    preload=False,
    lag=1,
    exp_split=True,
    s_only=False,
    u_only=False,
    staged_exp=False,
    dma_split=False,
    imm_bias=False,
):
    if staged_exp:
        lag = max(lag, 2)
    import contextlib

    import concourse.mybir as mybir
    import concourse.tile as tile
    from concourse import bacc
    from concourse.bass import ds, ts

    f32 = mybir.dt.float32
    f32r = mybir.dt.float32r
    bf16 = mybir.dt.bfloat16

    nc = bacc.Bacc("TRN2", target_bir_lowering=False)
    p_in = nc.dram_tensor("p_in", [C, N], f32r, kind="ExternalInput")
    q_in = nc.dram_tensor("q_in", [C, N], f32r, kind="ExternalInput")
    if BIGDESC:
        z_in = nc.dram_tensor("z_in", [P, NT, C + 2], bf16, kind="ExternalInput")
        out_t = nc.dram_tensor("out_t", [P, N // P, C], f32, kind="ExternalOutput")
    else:
        z_in = nc.dram_tensor("z_in", [N, C + 2], bf16, kind="ExternalInput")
        out_t = nc.dram_tensor("out_t", [N, C], f32, kind="ExternalOutput")

    ZG = 4  # z-load granularity (nt tiles per DMA)

    with tile.TileContext(nc) as tc:
        with (
            tc.tile_pool(name="big", bufs=1) as big,
            tc.tile_pool(name="epool", bufs=4) as epool,
            tc.tile_pool(name="opool", bufs=3) as opool,
            tc.tile_pool(name="small", bufs=4) as small,
            tc.tile_pool(name="spsum", bufs=2, space="PSUM") as spsum,
            tc.tile_pool(name="upsum", bufs=4, space="PSUM") as upsum,
        ):
            # Resident inputs.  p/q: [d, l|r] as [128, 2, N]; z: [l, c+pad]
            # as [128, NT, C+2] with two ones-columns (denominator + fp32r
            # even-width padding).  q and z are loaded in slices so the
            # first matmuls don't wait for the full 12 MB of input.
            p_t = big.tile([P, 2, N], f32r, tag="p", name="p_t")
            q_t = big.tile([P, 2, N], f32r, tag="q", name="q_t")
            z_t = big.tile([P, NT, C + 2], bf16, tag="z", name="z_t")
            kbias = small.tile([P, 1], f32, tag="kbias", name="kbias")
            nc.vector.memset(kbias[:], -KSHIFT)
            if u_only:
                e2c = big.tile([P, 2, MB], bf16, tag="e2c", name="e2c")
                nc.vector.memset(e2c[:], 0.001)

            p_src = p_in.rearrange("(ko p) n -> p ko n", p=P)
            q_src = q_in.rearrange("(ko p) n -> p ko n", p=P)
            if BIGDESC:
                z_src = z_in
            else:
                z_src = z_in.rearrange("(nt p) c -> p nt c", p=P)

            if loop_trip is not None:
                rep_ctx = lambda: tc.For_i(  # noqa: E731
                    0,
                    loop_trip,
                    1,
                    hint_engines=(
                        mybir.EngineType.PE,
                        mybir.EngineType.Activation,
                        mybir.EngineType.DVE,
                        mybir.EngineType.SP,
                    ),
                )
            else:
                rep_ctx = contextlib.nullcontext

            # issue order: what the first matmuls need comes first.
            # dma_split: q+z ride the Act HWDGE queue (issued at the
            # iteration head where their waits are already satisfied, so
            # they never stall the ACT compute stream); p + out stores
            # stay on the SP queue.
            ldq_eng = nc.scalar if dma_split else nc.sync

            def load_p(pg):
                nc.sync.dma_start(
                    p_t[:, :, ts(pg, N // 4)], p_src[:, :, ts(pg, N // 4)]
                )

            def load_q(mbq):
                ldq_eng.dma_start(q_t[:, :, ts(mbq, MB)], q_src[:, :, ts(mbq, MB)])

            def load_z(zg):
                ldq_eng.dma_start(z_t[:, ts(zg, ZG), :], z_src[:, ts(zg, ZG), :])

            def load_all():
                if BIGDESC:
                    # few DMAs with multi-KB per-partition descriptors; the
                    # head slices are small so the first matmuls start early
                    nc.sync.dma_start(p_t[:, :, ts(0, 512)], p_src[:, :, ts(0, 512)])
                    nc.sync.dma_start(q_t[:, :, ts(0, 512)], q_src[:, :, ts(0, 512)])
                    nc.sync.dma_start(z_t[:, 0:2, :], z_src[:, 0:2, :])
                    nc.sync.dma_start(
                        p_t[:, :, ds(512, N - 512)], p_src[:, :, ds(512, N - 512)]
                    )
                    for lo, sz in ((512, 1024), (1536, 1024), (2560, 1536)):
                        nc.sync.dma_start(
                            q_t[:, :, ds(lo, sz)], q_src[:, :, ds(lo, sz)]
                        )
                    for lo, sz in ((2, 6), (8, 12), (20, 12)):
                        nc.sync.dma_start(
                            z_t[:, ds(lo, sz), :], z_src[:, ds(lo, sz), :]
                        )
                    return
                load_p(0)
                load_q(0)
                load_z(0)
                for pg in range(1, 4):
                    load_p(pg)
                for i in range(1, MBS):
                    load_q(i)
                    load_z(i)

            if preload:
                load_all()

            def emit_body():
                if not preload:
                    load_all()

                for mb in range(MBS):
                    u_ps = [
                        upsum.tile([P, C + 2], mybir.dt.float32, tag="u", name=f"u{j}")
                        for j in range(u_j)
                    ]

                    def u_matmuls(nt2, e2_t):
                        for h in range(2):
                            nt = 2 * nt2 + h
                            for j in range(u_j):
                                nc.tensor.matmul(
                                    u_ps[j][:],
                                    e2_t[:, h, ts(j, P)],
                                    z_t[:, nt, :],
                                    start=(nt == 0),
                                    stop=(nt == NT - 1),
                                )

                    if s_only:
                        # probe: pure S-MM stream; a tiny DVE copy + DMA
                        # keeps each s tile live without ACT/U work
                        for nt2 in range(NT // 2):
                            s_ps = spsum.tile(
                                [P, 2, MB], mybir.dt.float32, tag="s", name="s"
                            )
                            for h in range(2):
                                nt = 2 * nt2 + h
                                for ko in range(2):
                                    nc.tensor.matmul(
                                        s_ps[:, h, :],
                                        p_t[:, ko, ts(nt, P)],
                                        q_t[:, ko, ts(mb, MB)],
                                        start=(ko == 0),
                                        stop=(ko == 1),
                                    )
                            o_t = opool.tile([P, 32], f32, tag="o", name="o")
                            nc.vector.tensor_copy(o_t[:], s_ps[:, 0, 0:32])
                            blk = (mb * (NT // 2) + nt2) % 32
                            if BIGDESC:
                                nc.sync.dma_start(out_t[:, blk, 0:32], o_t[:])
                            else:
                                nc.sync.dma_start(
                                    out_t[ds(blk * P, P), 0:32], o_t[:]
                                )
                        continue

                    if u_only:
                        # probe: pure U-MM stream off a constant E tile
                        for nt2 in range(NT // 2):
                            u_matmuls(nt2, e2c)
                        o4_t = opool.tile([P, u_j, C], f32, tag="o", name="o")
                        for j in range(u_j):
                            recip = small.tile([P, 1], f32, tag="recip", name="recip")
                            nc.vector.reciprocal(recip[:], u_ps[j][:, C : C + 1])
                            nc.vector.tensor_scalar_mul(
                                o4_t[:, j, :], u_ps[j][:, 0:C], recip[:]
                            )
                        if BIGDESC:
                            nc.sync.dma_start(
                                out_t[:, ds(mb * u_j, u_j), :], o4_t[:]
                            )
                        else:
                            for j in range(u_j):
                                nc.sync.dma_start(
                                    out_t[ds(mb * MB + j * P, P), :], o4_t[:, j, :]
                                )
                        continue

                    # Two S tiles share one PSUM tile (2 banks) so a single
                    # ACT instruction computes exp over 1024 elems/partition,
                    # halving ACT fixed overhead.  Software-pipelined with a
                    # lag of 2: PE runs S(nt2) before U(nt2-2), giving each
                    # exp two full PE iterations of slack so the U matmuls
                    # never wait on ACT.
                    pending = []
                    for nt2 in range(NT // 2):
                        s_ps = spsum.tile(
                            [P, 2, MB], mybir.dt.float32, tag="s", name="s"
                        )
                        for h in range(2):
                            nt = 2 * nt2 + h
                            for ko in range(2):
                                nc.tensor.matmul(
                                    s_ps[:, h, :],
                                    p_t[:, ko, ts(nt, P)],
                                    q_t[:, ko, ts(mb, MB)],
                                    start=(ko == 0),
                                    stop=(ko == 1),
                                )
                        e2_t = epool.tile([P, 2, MB], bf16, tag="e", name="e")
                        if exp_split:
                            for h in range(2):
                                nc.scalar.activation(
                                    e2_t[:, h, :],
                                    s_ps[:, h, :],
                                    mybir.ActivationFunctionType.Exp,
                                    bias=(-KSHIFT) if imm_bias else kbias[:],
                                )
                        else:
                            nc.scalar.activation(
                                e2_t[:],
                                s_ps[:],
                                mybir.ActivationFunctionType.Exp,
                                bias=kbias[:],
                            )
                        pending.append((nt2, e2_t))
                        if len(pending) > lag:
                            u_matmuls(*pending.pop(0))
                    for args in pending:
                        u_matmuls(*args)

                    if BIGDESC:
                        o4_t = opool.tile([P, u_j, C], f32, tag="o", name="o")
                        for j in range(u_j):
                            recip = small.tile([P, 1], f32, tag="recip", name="recip")
                            nc.vector.reciprocal(recip[:], u_ps[j][:, C : C + 1])
                            nc.vector.tensor_scalar_mul(
                                o4_t[:, j, :], u_ps[j][:, 0:C], recip[:]
                            )
                        nc.sync.dma_start(
                            out_t[:, ds(mb * u_j, u_j), :], o4_t[:]
                        )
                    else:
                        for j in range(u_j):
                            recip = small.tile([P, 1], f32, tag="recip", name="recip")
                            nc.vector.reciprocal(recip[:], u_ps[j][:, C : C + 1])
                            o_t = opool.tile([P, C], f32, tag="o", name="o")
                            nc.vector.tensor_scalar_mul(
                                o_t[:], u_ps[j][:, 0:C], recip[:]
                            )
                            nc.sync.dma_start(
                                out_t[ds(mb * MB + j * P, P), :], o_t[:]
                            )

            if loop_trip is not None:
                with rep_ctx():
                    for _rep in range(reps):
                        emit_body()
            else:
                for _rep in range(reps):
                    emit_body()

    nc.compile()
    return nc


def _get_nc(reps=1, loop_trip=None, **bkw):
    key = f"nc{reps}_{loop_trip}_{sorted(bkw.items())}"
    if key not in _STATE:
        _STATE[key] = _build(reps, loop_trip, **bkw)
    return _STATE[key]


def _with_ones(x):
    import ml_dtypes

    z = np.ones((N, C + 2), dtype=ml_dtypes.bfloat16)
    z[:, 0:C] = x.T.astype(ml_dtypes.bfloat16)
    if BIGDESC:
        # partition-major: z[p, nt, c] = z_rowmajor[nt*128 + p, c]
        z = np.ascontiguousarray(z.reshape(NT, P, C + 2).transpose(1, 0, 2))
    return z


def _prep_inputs(a, b, W):
    a = np.asarray(a, dtype=np.float32)
    b = np.asarray(b, dtype=np.float32)
    W = np.asarray(W, dtype=np.float32)
    B = a.shape[0]
    af = a.reshape(B, C, N)
    bf = b.reshape(B, C, N)
    Wa = np.matmul(W[None], af)  # [B, C, N]
    in_maps = []
    for i in range(B):  # a-cores
        in_maps.append(
            {
                "p_in": np.ascontiguousarray(Wa[i]),
                "q_in": np.ascontiguousarray(bf[i]),
                "z_in": _with_ones(af[i]),
            }
        )
    for i in range(B):  # b-cores
        in_maps.append(
            {
                "p_in": np.ascontiguousarray(bf[i]),
                "q_in": np.ascontiguousarray(Wa[i]),
                "z_in": _with_ones(bf[i]),
            }
        )
    return in_maps


def _unout(r):
    # [P, N//P, C] partition-major -> [N, C] row-major -> [C, N]
    if BIGDESC:
        return r.swapaxes(0, 1).reshape(N, C).T
    return r.T


def _postprocess(results, B):
    a_new = np.stack([_unout(results[i]["out_t"]) for i in range(B)])
    b_new = np.stack([_unout(results[B + i]["out_t"]) for i in range(B)])
    a_new = np.ascontiguousarray(a_new).reshape(B, C, *HW_SHAPE)
    b_new = np.ascontiguousarray(b_new).reshape(B, C, *HW_SHAPE)
    return a_new, b_new


def _run(a, b, W, reps=1, loop_trip=None, **run_kwargs):
    from concourse.bass_utils import run_bass_kernel_spmd

    in_maps = _prep_inputs(a, b, W)
    nc = _get_nc(reps, loop_trip)
    res = run_bass_kernel_spmd(nc, in_maps, core_ids=list(range(len(in_maps))), **run_kwargs)
    return _postprocess(res.results, len(in_maps) // 2), res


def kernel(a, b, W):
    (a_new, b_new), _ = _run(a, b, W)
    return a_new, b_new

